# revision 12
# baseline (speedup 1.0000x reference)
"""Trainium2 kernel for MinibatchDiscrimination.

reference:
    M = einsum('ni,ibk->nbk', x, T)            # (256, 256, 16)
    l1[n,m,b] = sum_k |M[n,b,k] - M[m,b,k]|
    out[m,b]  = sum_n exp(-l1[n,m,b]) - 1      # (256, 256)
    return concat([x, out], axis=1)            # (256, 1280)

Sharding: tensor-parallel over the B_extra=256 feature dim -> 32 features
per core, no collectives. Each core computes out[:, shard] as [32, 256]
(batch on partitions), host transposes and concatenates with x.

Per-core dataflow (block-triangle; E[n,m] symmetric):
  MT[(b,k), m] = M[m, b, k] via PE fp32r matmuls, (b,k) in 4 chunks of 128.
  Groups of p rows starting at n0 (p ~ 512/w, w = 256-n0) cover rows
  [n0, n0+p) x cols [n0, 256). Per group, one of two diff modes chosen by
  a static greedy balance over the engine cost model:
    abs mode: |M[:, m] - M[:, n]| per (row, chunk) -- ONE dual-op
        tensor_scalar (subtract, abs_max) at 4x on DVE, or an Abs
        activation (per-partition bias) on ACT; 4 selector matmuls (W)
        contract the (b,k) partitions into PSUM = exact l1.
    relu mode (GPSIMD): relu(d) via TT-subtract + TS-max (the only ALU
        ops walrus allows on Pool); PSUM pre-seeded with SS[n] - SS[m]
        (two fp32r matmuls with step-0 moving APs; SS = selector-contracted
        M, exact in f32) and selectors use W2 = 2W, since
        sum|d| = 2*sum relu(d) - SS[m] + SS[n].
  ACT: ONE exp per group over the whole [32, p, w] block (bias-free).
  PE: p identity passthrough-matmuls accumulate column sums into a
      PSUM-resident acc (memset to -1 cancels the diagonal's exp(0)=1
      against the final "-1").
  DVE: one tensor_reduce per group row-sums the beyond-the-pack columns
      into accn[:, n0:n0+p] (the mirrored cross-group triangle half).
  out_dev[b, m] = acc + accn, DMA'd [32, 256].

Each unordered pair lands exactly once: n<m via the colsum of n's group,
n>m cross-group via the rowsum of m's group (E symmetric), intra-pack
pairs both orders via the colsum of the shared group.
"""

import sys

sys.path.insert(0, "/opt/trn_rl_repo")

import os
import numpy as np
import ml_dtypes

LAG = int(os.environ.get("MBD_LAG", "3"))
R_BUFS = int(os.environ.get("MBD_R_BUFS", "16"))
E_BUFS = int(os.environ.get("MBD_E_BUFS", "6"))
PSL1_BUFS = int(os.environ.get("MBD_PSL1_BUFS", "4"))
PSMT_BUFS = int(os.environ.get("MBD_PSMT_BUFS", "2"))
# cost-model weights (ns) for the static assignment; tune empirically
C_DVE_I = float(os.environ.get("MBD_C_DVE_I", "60.4"))
C_DVE_W = float(os.environ.get("MBD_C_DVE_W", "0.26"))
C_ACT_I = float(os.environ.get("MBD_C_ACT_I", "225"))
C_ACT_W = float(os.environ.get("MBD_C_ACT_W", "0.833"))
C_POOL_I = float(os.environ.get("MBD_C_POOL_I", "700"))
C_POOL_W = float(os.environ.get("MBD_C_POOL_W", "12.0"))
DVE_FIXED = float(os.environ.get("MBD_DVE_FIXED", "4000"))
ACT_FIXED = float(os.environ.get("MBD_ACT_FIXED", "0"))
POOL_FIXED = float(os.environ.get("MBD_POOL_FIXED", "0"))
PE_FIXED = float(os.environ.get("MBD_PE_FIXED", "4500"))

N = 256
IN_FEATURES = 1024
B_EXTRA = 256
K = 16
N_CORES = 8
B_LOCAL = B_EXTRA // N_CORES          # 32 features per core
BK = B_LOCAL * K                      # 512 = (b_local, k) flattened
N_CHUNKS = BK // 128                  # 4 partition chunks of (b,k)
I_CHUNKS = IN_FEATURES // 128         # 8 contraction chunks

_COMPILED = None


def _apply_tile_drain_patch():
    """walrus in this container caps Drain (CTRL) instructions at one sem
    wait; Tile's end-of-kernel drain carries one wait per outstanding proc.
    Split the waits across a chain of drains."""
    from concourse import mybir, tile
    from concourse.vector_clock import ScopedClock

    def _drain_and_barrier(self, tick_clock, wait_clock):
        drain_inst = self.nc.sync.drain()
        wait_clock.add_sem_waits(
            drain_inst.ins, ScopedClock({None: tick_clock.global_clock})
        )
        si = drain_inst.ins.sync_info
        if si is not None and si.on_wait and len(si.on_wait) > 1:
            waits = list(si.on_wait)
            drain_inst.ins.sync_info = mybir.SyncInfo(
                on_wait=[waits[0]], on_update=list(si.on_update or [])
            )
            for w in waits[1:]:
                d = self.nc.sync.drain()
                d.ins.sync_info = mybir.SyncInfo(on_wait=[w], on_update=[])

        self.nc.all_engine_barrier()
        assert self.sems is not None
        popped = self.nc._tile_sem_poison_stack.pop()
        assert popped is self._sem_poison
        self.nc.clear_and_free_semaphores(list(self.sems.allocated().values()))
        self.nc.all_engine_barrier()

    tile.TileContext._drain_and_barrier = _drain_and_barrier


def _split_multi_waits(nc, max_waits=1):
    """This walrus build accepts at most one sync wait per instruction.
    Hoist extra waits onto NoOp instructions inserted just before the
    offending instruction in the same engine's stream."""
    from concourse import mybir

    cnt = 0
    for blk in nc.main_func.blocks:
        insts = blk.instructions
        if not any(
            inst.sync_info is not None
            and inst.sync_info.on_wait
            and len(inst.sync_info.on_wait) > max_waits
            for inst in insts
        ):
            continue
        new_list = []
        for inst in insts:
            si = inst.sync_info
            if si is not None and si.on_wait and len(si.on_wait) > max_waits:
                waits = list(si.on_wait)
                for w in waits[:-max_waits]:
                    nop = mybir.InstNoOp(name=f"wsplit-{cnt}", ins=[], outs=[])
                    cnt += 1
                    nop.engine = inst.engine
                    nop.sync_info = mybir.SyncInfo(on_wait=[w], on_update=[])
                    new_list.append(nop)
                inst.sync_info = mybir.SyncInfo(
                    on_wait=waits[-max_waits:],
                    on_update=list(si.on_update or []),
                )
            new_list.append(inst)
        insts[:] = new_list
    return cnt


def _groups():
    gs = []
    n0 = 0
    while n0 < N:
        w = N - n0
        p = 2 * (256 // w)
        p = min(p, w)
        gs.append((n0, p, w))
        n0 += p
    return gs


def _assign_diffs(groups):
    """Per-(group, chunk) diff engine choice, greedily minimizing the
    projected makespan over engine loads (ns). All groups use the relu
    formulation (sum|d| = 2*sum relu(d) + SS[n] - SS[m]); walrus has no
    abs op on DVE/Pool, and uniform relu keeps the PSUM seed unmasked."""
    loads = {
        "dve": DVE_FIXED + sum(
            (58 + p * (w - p)) * 1.0417 for (_, p, w) in groups if w > p),
        "act": ACT_FIXED + sum(
            210.0 + 0.833 * p * w for (_, p, w) in groups),
        "pool": POOL_FIXED,
        "pe": PE_FIXED + sum(7 * p * w * 0.4167 for (_, p, w) in groups),
    }
    cost = {
        "dve": lambda p, w: p * (C_DVE_I + C_DVE_W * w),
        "act": lambda p, w: p * (C_ACT_I + C_ACT_W * w),
        "pool": lambda p, w: p * (C_POOL_I + C_POOL_W * w),
    }
    plans = []
    for (n0, p, w) in groups:
        chunks = []
        for c in range(N_CHUNKS):
            best = min(cost, key=lambda e: loads[e] + cost[e](p, w))
            loads[best] += cost[best](p, w)
            chunks.append(best)
        plans.append(chunks)
    return plans, loads


def _build():
    from concourse import bass, mybir, tile

    _apply_tile_drain_patch()
    A = mybir.AluOpType
    F32 = mybir.dt.float32
    F32R = mybir.dt.float32r
    BF16 = mybir.dt.bfloat16

    nc = bass.Bass()
    xt_d = nc.declare_dram_parameter("xT", [IN_FEATURES, N], F32R, isOutput=False)
    t_d = nc.declare_dram_parameter("Tsh", [IN_FEATURES, BK], F32R, isOutput=False)
    w_d = nc.declare_dram_parameter("W", [128, N_CHUNKS * B_LOCAL], BF16,
                                    isOutput=False)
    w2_d = nc.declare_dram_parameter("W2", [128, N_CHUNKS * B_LOCAL], BF16,
                                     isOutput=False)
    i_d = nc.declare_dram_parameter("I32", [B_LOCAL, B_LOCAL], BF16,
                                    isOutput=False)
    ir_d = nc.declare_dram_parameter("I32R", [B_LOCAL, B_LOCAL], F32R,
                                     isOutput=False)
    out_d = nc.declare_dram_parameter("out", [B_LOCAL, N], F32, isOutput=True)

    groups = _groups()
    plans, loads = _assign_diffs(groups)

    Exp = mybir.ActivationFunctionType.Exp
    Relu = mybir.ActivationFunctionType.Relu

    with tile.TileContext(nc) as tc:
        with (
            tc.tile_pool(name="const", bufs=1) as const_pool,
            tc.tile_pool(name="mt", bufs=1) as mt_pool,
            tc.tile_pool(name="r", bufs=R_BUFS) as r_pool,
            tc.tile_pool(name="tmp", bufs=4) as tmp_pool,
            tc.tile_pool(name="e", bufs=E_BUFS) as e_pool,
            tc.tile_pool(name="psmt", bufs=PSMT_BUFS, space="PSUM") as psmt_pool,
            tc.tile_pool(name="psss", bufs=1, space="PSUM") as psss_pool,
            tc.tile_pool(name="psl1", bufs=PSL1_BUFS, space="PSUM") as psl1_pool,
            tc.tile_pool(name="psacc", bufs=1, space="PSUM") as psacc_pool,
        ):
            # ---- load inputs (split so MT matmuls can overlap the DMA) ----
            xt = const_pool.tile([128, I_CHUNKS, N], F32R, tag="xt")
            for ic in range(I_CHUNKS):
                nc.sync.dma_start(
                    xt[:, ic, :], xt_d[128 * ic:128 * (ic + 1), :])
            tsh = const_pool.tile([128, I_CHUNKS, BK], F32R, tag="tsh")
            for ic in range(I_CHUNKS):
                nc.sync.dma_start(
                    tsh[:, ic, :], t_d[128 * ic:128 * (ic + 1), :])
            w_sb = const_pool.tile([128, N_CHUNKS * B_LOCAL], BF16, tag="w")
            nc.sync.dma_start(w_sb[:], w_d[:])
            w2_sb = const_pool.tile([128, N_CHUNKS * B_LOCAL], BF16, tag="w2")
            nc.sync.dma_start(w2_sb[:], w2_d[:])
            i_sb = const_pool.tile([B_LOCAL, B_LOCAL], BF16, tag="i32")
            nc.sync.dma_start(i_sb[:], i_d[:])
            ir_sb = const_pool.tile([B_LOCAL, B_LOCAL], F32R, tag="i32r")
            nc.sync.dma_start(ir_sb[:], ir_d[:])

            # ---- MT[(b,k), m] per chunk: bf16 stream + f32 scalar copies ----
            # mt_f = f32(upcast(bf16(M))) so the diff diagonal is exactly 0.
            mt_b, mt_f, mtn_f = [], [], []
            for c in range(N_CHUNKS):
                ps = psmt_pool.tile([128, N], F32)
                for ic in range(I_CHUNKS):
                    nc.tensor.matmul(
                        ps[:],
                        tsh[:, ic, 128 * c:128 * (c + 1)],
                        xt[:, ic, :],
                        start=(ic == 0),
                        stop=(ic == I_CHUNKS - 1),
                    )
                mb = mt_pool.tile([128, N], BF16, tag=f"mtb{c}")
                nc.vector.tensor_copy(mb[:], ps[:])
                mf = mt_pool.tile([128, N], F32, tag=f"mtf{c}")
                nc.vector.tensor_copy(mf[:], mb[:])
                nf = mt_pool.tile([128, N], F32, tag=f"mtnf{c}")
                nc.vector.tensor_scalar(nf[:], mb[:], -1.0, None, A.mult)
                mt_b.append(mb)
                mt_f.append(mf)
                mtn_f.append(nf)

            # ---- SS[b, m] = sum_k bf16(M)[m, b, k] for the l1 seeds ----
            if True:
                ss_ps = psss_pool.tile([B_LOCAL, N], F32, tag="ssps")
                for c in range(N_CHUNKS):
                    nc.tensor.matmul(
                        ss_ps[:], w_sb[:, B_LOCAL * c:B_LOCAL * (c + 1)],
                        mt_b[c][:], start=(c == 0), stop=(c == N_CHUNKS - 1))
                ss_pos = mt_pool.tile([B_LOCAL, N], F32R, tag="sspos")
                nc.vector.tensor_copy(ss_pos[:], ss_ps[:])
                ss_neg = mt_pool.tile([B_LOCAL, N], F32R, tag="ssneg")
                nc.vector.tensor_scalar(
                    ss_neg[:], ss_ps[:], -1.0, None, A.mult)

            # ---- accumulators ----
            accn = e_pool.tile([B_LOCAL, N], F32, tag="accn")
            nc.gpsimd.memset(accn[:], 0.0)
            acc_ps = psacc_pool.tile([B_LOCAL, N], F32)
            nc.vector.memset(acc_ps[:], -1.0)   # cancels the diagonal exp(0)=1

            # ---- main loop ----
            pending = []                        # (e, n0, p, w) awaiting sums

            def flush_one():
                e_t, n0, p, w = pending.pop(0)
                for j in range(p):
                    nc.tensor.matmul(
                        acc_ps[:, n0:N], i_sb[:], e_t[:, j, :],
                        start=False, stop=False, skip_group_check=True)
                if w > p:
                    nc.vector.tensor_reduce(
                        accn[:, n0:n0 + p], e_t[:, :, p:w],
                        mybir.AxisListType.X, A.add)

            for gi, (n0, p, w) in enumerate(groups):
                chunks = plans[gi]
                ps = psl1_pool.tile([B_LOCAL, p, w], F32)
                # seed l1 with SS[n] - SS[m]; selectors add 2*sum relu(d)
                nc.tensor.matmul(
                    ps[:], ir_sb[:],
                    ss_pos[:, n0:n0 + p].rearrange(
                        "b p -> b p ()").broadcast_to((B_LOCAL, p, w)),
                    start=True, stop=False)
                nc.tensor.matmul(
                    ps[:], ir_sb[:],
                    ss_neg[:, n0:N].rearrange(
                        "b w -> b () w").broadcast_to((B_LOCAL, p, w)),
                    start=False, stop=False)
                for c in range(N_CHUNKS):
                    eng = chunks[c]
                    r = r_pool.tile([128, p, w], BF16, tag="r")
                    for j in range(p):
                        n = n0 + j
                        if eng == "dve":
                            nc.vector.tensor_scalar(
                                r[:, j, :], mt_b[c][:, n0:N],
                                mt_f[c][:, n:n + 1], 0.0,
                                A.subtract, A.max)
                        elif eng == "act":
                            nc.scalar.activation(
                                r[:, j, :], mt_b[c][:, n0:N], Relu,
                                bias=mtn_f[c][:, n:n + 1], scale=1.0)
                        else:
                            bc = mt_b[c][:, n:n + 1].broadcast_to((128, w))
                            tmp = tmp_pool.tile([128, w], BF16, tag="tmp")
                            nc.gpsimd.tensor_tensor(
                                tmp[:], mt_b[c][:, n0:N], bc, A.subtract)
                            nc.gpsimd.tensor_scalar(
                                r[:, j, :], tmp[:], 0.0, None, A.max)
                    nc.tensor.matmul(
                        ps[:], w2_sb[:, B_LOCAL * c:B_LOCAL * (c + 1)],
                        r[:], start=False, stop=(c == N_CHUNKS - 1))
                e = e_pool.tile([B_LOCAL, p, w], BF16, tag="e")
                nc.scalar.activation(e[:], ps[:], Exp, bias=0.0, scale=-1.0)
                pending.append((e, n0, p, w))
                if len(pending) > LAG:
                    flush_one()
            while pending:
                flush_one()

            # ---- combine halves and store ----
            accf = e_pool.tile([B_LOCAL, N], F32, tag="accf")
            nc.vector.tensor_tensor(accf[:], accn[:], acc_ps[:], A.add)
            nc.sync.dma_start(out_d[:], accf[:])

    _split_multi_waits(nc)
    return nc


def _selector(scale: float) -> np.ndarray:
    w = np.zeros((128, N_CHUNKS, B_LOCAL), dtype=np.float32)
    for c in range(N_CHUNKS):
        for p in range(128):
            w[p, c, (128 * c + p) // K] = scale
    return w.reshape(128, N_CHUNKS * B_LOCAL).astype(ml_dtypes.bfloat16)


def _in_maps(x: np.ndarray, T: np.ndarray) -> list:
    xt = np.ascontiguousarray(x.T)                       # (1024, 256)
    w = _selector(1.0)
    w2 = _selector(2.0)
    eye = np.eye(B_LOCAL, dtype=np.float32).astype(ml_dtypes.bfloat16)
    in_maps = []
    for c in range(N_CORES):
        tsh = np.ascontiguousarray(
            T[:, c * B_LOCAL:(c + 1) * B_LOCAL, :].reshape(IN_FEATURES, BK))
        in_maps.append({"xT": xt, "Tsh": tsh, "W": w, "W2": w2, "I32": eye,
                        "I32R": np.eye(B_LOCAL, dtype=np.float32)})
    return in_maps


def kernel(x: np.ndarray, T: np.ndarray) -> np.ndarray:
    global _COMPILED
    from concourse.bass_utils import run_bass_kernel_spmd

    x = np.ascontiguousarray(x, dtype=np.float32)
    T = np.ascontiguousarray(T, dtype=np.float32)

    if _COMPILED is None:
        _COMPILED = _build()
    nc = _COMPILED

    res = run_bass_kernel_spmd(nc, _in_maps(x, T), core_ids=list(range(N_CORES)))

    out = np.empty((N, IN_FEATURES + B_EXTRA), dtype=np.float32)
    out[:, :IN_FEATURES] = x
    for c in range(N_CORES):
        blk = res.results[c]["out"]                      # (32, 256) = (b, m)
        out[:, IN_FEATURES + c * B_LOCAL:IN_FEATURES + (c + 1) * B_LOCAL] = blk.T
    return out


# revision 14
# speedup vs baseline: 1.1517x; 1.1517x over previous
"""Trainium2 kernel for MinibatchDiscrimination.

reference:
    M = einsum('ni,ibk->nbk', x, T)            # (256, 256, 16)
    l1[n,m,b] = sum_k |M[n,b,k] - M[m,b,k]|
    out[m,b]  = sum_n exp(-l1[n,m,b]) - 1      # (256, 256)
    return concat([x, out], axis=1)            # (256, 1280)

Sharding: tensor-parallel over the B_extra=256 feature dim -> 32 features
per core, no collectives. Each core computes out[:, shard] as [32, 256]
(batch on partitions), host transposes and concatenates with x.

Per-core dataflow (block-triangle; E[n,m] symmetric):
  MT[(b,k), m] = M[m, b, k] via PE fp32r matmuls, (b,k) in 4 chunks of 128.
  Groups of p rows starting at n0 (p ~ 512/w, w = 256-n0) cover rows
  [n0, n0+p) x cols [n0, 256). Per group, one of two diff modes chosen by
  a static greedy balance over the engine cost model:
    abs mode: |M[:, m] - M[:, n]| per (row, chunk) -- ONE dual-op
        tensor_scalar (subtract, abs_max) at 4x on DVE, or an Abs
        activation (per-partition bias) on ACT; 4 selector matmuls (W)
        contract the (b,k) partitions into PSUM = exact l1.
    relu mode (GPSIMD): relu(d) via TT-subtract + TS-max (the only ALU
        ops walrus allows on Pool); PSUM pre-seeded with SS[n] - SS[m]
        (two fp32r matmuls with step-0 moving APs; SS = selector-contracted
        M, exact in f32) and selectors use W2 = 2W, since
        sum|d| = 2*sum relu(d) - SS[m] + SS[n].
  ACT: ONE exp per group over the whole [32, p, w] block (bias-free).
  PE: p identity passthrough-matmuls accumulate column sums into a
      PSUM-resident acc (memset to -1 cancels the diagonal's exp(0)=1
      against the final "-1").
  DVE: one tensor_reduce per group row-sums the beyond-the-pack columns
      into accn[:, n0:n0+p] (the mirrored cross-group triangle half).
  out_dev[b, m] = acc + accn, DMA'd [32, 256].

Each unordered pair lands exactly once: n<m via the colsum of n's group,
n>m cross-group via the rowsum of m's group (E symmetric), intra-pack
pairs both orders via the colsum of the shared group.
"""

import sys

sys.path.insert(0, "/opt/trn_rl_repo")

import os
import numpy as np
import ml_dtypes

LAG = int(os.environ.get("MBD_LAG", "4"))
R_BUFS = int(os.environ.get("MBD_R_BUFS", "32"))
E_BUFS = int(os.environ.get("MBD_E_BUFS", "8"))
PSL1_BUFS = int(os.environ.get("MBD_PSL1_BUFS", "6"))
PSMT_BUFS = int(os.environ.get("MBD_PSMT_BUFS", "2"))
# cost-model weights (ns) for the static assignment; tune empirically
C_DVE_I = float(os.environ.get("MBD_C_DVE_I", "60.4"))
C_DVE_W = float(os.environ.get("MBD_C_DVE_W", "0.26"))
C_ACT_I = float(os.environ.get("MBD_C_ACT_I", "225"))
C_ACT_W = float(os.environ.get("MBD_C_ACT_W", "0.833"))
C_POOL_I = float(os.environ.get("MBD_C_POOL_I", "700"))
C_POOL_W = float(os.environ.get("MBD_C_POOL_W", "1e6"))
DVE_FIXED = float(os.environ.get("MBD_DVE_FIXED", "4000"))
ACT_FIXED = float(os.environ.get("MBD_ACT_FIXED", "0"))
POOL_FIXED = float(os.environ.get("MBD_POOL_FIXED", "0"))
PE_FIXED = float(os.environ.get("MBD_PE_FIXED", "4500"))

N = 256
IN_FEATURES = 1024
B_EXTRA = 256
K = 16
N_CORES = 8
B_LOCAL = B_EXTRA // N_CORES          # 32 features per core
BK = B_LOCAL * K                      # 512 = (b_local, k) flattened
N_CHUNKS = BK // 128                  # 4 partition chunks of (b,k)
I_CHUNKS = IN_FEATURES // 128         # 8 contraction chunks

_COMPILED = None


def _apply_tile_drain_patch():
    """walrus in this container caps Drain (CTRL) instructions at one sem
    wait; Tile's end-of-kernel drain carries one wait per outstanding proc.
    Split the waits across a chain of drains."""
    from concourse import mybir, tile
    from concourse.vector_clock import ScopedClock

    def _drain_and_barrier(self, tick_clock, wait_clock):
        drain_inst = self.nc.sync.drain()
        wait_clock.add_sem_waits(
            drain_inst.ins, ScopedClock({None: tick_clock.global_clock})
        )
        si = drain_inst.ins.sync_info
        if si is not None and si.on_wait and len(si.on_wait) > 1:
            waits = list(si.on_wait)
            drain_inst.ins.sync_info = mybir.SyncInfo(
                on_wait=[waits[0]], on_update=list(si.on_update or [])
            )
            for w in waits[1:]:
                d = self.nc.sync.drain()
                d.ins.sync_info = mybir.SyncInfo(on_wait=[w], on_update=[])

        self.nc.all_engine_barrier()
        assert self.sems is not None
        popped = self.nc._tile_sem_poison_stack.pop()
        assert popped is self._sem_poison
        self.nc.clear_and_free_semaphores(list(self.sems.allocated().values()))
        self.nc.all_engine_barrier()

    tile.TileContext._drain_and_barrier = _drain_and_barrier


def _split_multi_waits(nc, max_waits=1):
    """This walrus build accepts at most one sync wait per instruction.
    Hoist extra waits onto NoOp instructions inserted just before the
    offending instruction in the same engine's stream."""
    from concourse import mybir

    cnt = 0
    for blk in nc.main_func.blocks:
        insts = blk.instructions
        if not any(
            inst.sync_info is not None
            and inst.sync_info.on_wait
            and len(inst.sync_info.on_wait) > max_waits
            for inst in insts
        ):
            continue
        new_list = []
        for inst in insts:
            si = inst.sync_info
            if si is not None and si.on_wait and len(si.on_wait) > max_waits:
                waits = list(si.on_wait)
                for w in waits[:-max_waits]:
                    nop = mybir.InstNoOp(name=f"wsplit-{cnt}", ins=[], outs=[])
                    cnt += 1
                    nop.engine = inst.engine
                    nop.sync_info = mybir.SyncInfo(on_wait=[w], on_update=[])
                    new_list.append(nop)
                inst.sync_info = mybir.SyncInfo(
                    on_wait=waits[-max_waits:],
                    on_update=list(si.on_update or []),
                )
            new_list.append(inst)
        insts[:] = new_list
    return cnt


def _groups():
    gs = []
    n0 = 0
    while n0 < N:
        w = N - n0
        p = 2 * (256 // w)
        p = min(p, w)
        gs.append((n0, p, w))
        n0 += p
    return gs


def _assign_diffs(groups):
    """Per-(group, chunk) diff engine choice, greedily minimizing the
    projected makespan over engine loads (ns). All groups use the relu
    formulation (sum|d| = 2*sum relu(d) + SS[n] - SS[m]); walrus has no
    abs op on DVE/Pool, and uniform relu keeps the PSUM seed unmasked."""
    loads = {
        "dve": DVE_FIXED + sum(
            (58 + p * (w - p)) * 1.0417 for (_, p, w) in groups if w > p),
        "act": ACT_FIXED + sum(
            210.0 + 0.833 * p * w for (_, p, w) in groups),
        "pool": POOL_FIXED,
        "pe": PE_FIXED + sum(7 * p * w * 0.4167 for (_, p, w) in groups),
    }
    cost = {
        "dve": lambda p, w: p * (C_DVE_I + C_DVE_W * w),
        "act": lambda p, w: p * (C_ACT_I + C_ACT_W * w),
        "pool": lambda p, w: p * (C_POOL_I + C_POOL_W * w),
    }
    plans = []
    for (n0, p, w) in groups:
        chunks = []
        for c in range(N_CHUNKS):
            best = min(cost, key=lambda e: loads[e] + cost[e](p, w))
            loads[best] += cost[best](p, w)
            chunks.append(best)
        plans.append(chunks)
    return plans, loads


def _build():
    from concourse import bass, mybir, tile

    _apply_tile_drain_patch()
    A = mybir.AluOpType
    F32 = mybir.dt.float32
    F32R = mybir.dt.float32r
    BF16 = mybir.dt.bfloat16

    nc = bass.Bass()
    xt_d = nc.declare_dram_parameter("xT", [IN_FEATURES, N], F32R, isOutput=False)
    t_d = nc.declare_dram_parameter("Tsh", [IN_FEATURES, BK], F32R, isOutput=False)
    w_d = nc.declare_dram_parameter("W", [128, N_CHUNKS * B_LOCAL], BF16,
                                    isOutput=False)
    w2_d = nc.declare_dram_parameter("W2", [128, N_CHUNKS * B_LOCAL], BF16,
                                     isOutput=False)
    i_d = nc.declare_dram_parameter("I32", [B_LOCAL, B_LOCAL], BF16,
                                    isOutput=False)
    ir_d = nc.declare_dram_parameter("I32R", [B_LOCAL, B_LOCAL], F32R,
                                     isOutput=False)
    out_d = nc.declare_dram_parameter("out", [B_LOCAL, N], F32, isOutput=True)

    groups = _groups()
    plans, loads = _assign_diffs(groups)

    Exp = mybir.ActivationFunctionType.Exp
    Relu = mybir.ActivationFunctionType.Relu

    with tile.TileContext(nc) as tc:
        with (
            tc.tile_pool(name="const", bufs=1) as const_pool,
            tc.tile_pool(name="mt", bufs=1) as mt_pool,
            tc.tile_pool(name="r", bufs=R_BUFS) as r_pool,
            tc.tile_pool(name="tmp", bufs=4) as tmp_pool,
            tc.tile_pool(name="e", bufs=E_BUFS) as e_pool,
        ):
            # ---- load inputs (split so MT matmuls can overlap the DMA) ----
            xt = const_pool.tile([128, I_CHUNKS, N], F32R, tag="xt")
            for ic in range(I_CHUNKS):
                nc.sync.dma_start(
                    xt[:, ic, :], xt_d[128 * ic:128 * (ic + 1), :])
            tsh = const_pool.tile([128, I_CHUNKS, BK], F32R, tag="tsh")
            for ic in range(I_CHUNKS):
                nc.sync.dma_start(
                    tsh[:, ic, :], t_d[128 * ic:128 * (ic + 1), :])
            w_sb = const_pool.tile([128, N_CHUNKS * B_LOCAL], BF16, tag="w")
            nc.sync.dma_start(w_sb[:], w_d[:])
            w2_sb = const_pool.tile([128, N_CHUNKS * B_LOCAL], BF16, tag="w2")
            nc.sync.dma_start(w2_sb[:], w2_d[:])
            i_sb = const_pool.tile([B_LOCAL, B_LOCAL], BF16, tag="i32")
            nc.sync.dma_start(i_sb[:], i_d[:])
            ir_sb = const_pool.tile([B_LOCAL, B_LOCAL], F32R, tag="i32r")
            nc.sync.dma_start(ir_sb[:], ir_d[:])

            mt_psum = tc.tile_pool(name="psmt", bufs=PSMT_BUFS, space="PSUM")
            psmt_pool = mt_psum.__enter__()
            ss_psum = tc.tile_pool(name="psss", bufs=1, space="PSUM")
            psss_pool = ss_psum.__enter__()
            # ---- MT[(b,k), m] per chunk: bf16 stream + f32 scalar copies ----
            # mt_f = f32(upcast(bf16(M))) so the diff diagonal is exactly 0.
            mt_b, mt_f, mtn_f = [], [], []
            for c in range(N_CHUNKS):
                ps = psmt_pool.tile([128, N], F32)
                for ic in range(I_CHUNKS):
                    nc.tensor.matmul(
                        ps[:],
                        tsh[:, ic, 128 * c:128 * (c + 1)],
                        xt[:, ic, :],
                        start=(ic == 0),
                        stop=(ic == I_CHUNKS - 1),
                    )
                mb = mt_pool.tile([128, N], BF16, tag=f"mtb{c}")
                nc.vector.tensor_copy(mb[:], ps[:])
                mf = mt_pool.tile([128, N], F32, tag=f"mtf{c}")
                nc.vector.tensor_copy(mf[:], mb[:])
                nf = mt_pool.tile([128, N], F32, tag=f"mtnf{c}")
                nc.vector.tensor_scalar(nf[:], mb[:], -1.0, None, A.mult)
                mt_b.append(mb)
                mt_f.append(mf)
                mtn_f.append(nf)

            # ---- SS[b, m] = sum_k bf16(M)[m, b, k] for the l1 seeds ----
            if True:
                ss_ps = psss_pool.tile([B_LOCAL, N], F32, tag="ssps")
                for c in range(N_CHUNKS):
                    nc.tensor.matmul(
                        ss_ps[:], w_sb[:, B_LOCAL * c:B_LOCAL * (c + 1)],
                        mt_b[c][:], start=(c == 0), stop=(c == N_CHUNKS - 1))
                ss_pos = mt_pool.tile([B_LOCAL, N], F32R, tag="sspos")
                nc.vector.tensor_copy(ss_pos[:], ss_ps[:])
                ss_neg = mt_pool.tile([B_LOCAL, N], F32R, tag="ssneg")
                nc.vector.tensor_scalar(
                    ss_neg[:], ss_ps[:], -1.0, None, A.mult)

            ss_psum.__exit__(None, None, None)
            mt_psum.__exit__(None, None, None)
            lp = tc.tile_pool(name="psl1", bufs=PSL1_BUFS, space="PSUM")
            psl1_pool = lp.__enter__()
            ap_ = tc.tile_pool(name="psacc", bufs=1, space="PSUM")
            psacc_pool = ap_.__enter__()
            # ---- accumulators ----
            accn = e_pool.tile([B_LOCAL, N], F32, tag="accn")
            nc.gpsimd.memset(accn[:], 0.0)
            acc_ps = psacc_pool.tile([B_LOCAL, N], F32)
            nc.vector.memset(acc_ps[:], -1.0)   # cancels the diagonal exp(0)=1

            # ---- main loop ----
            pending = []                        # (e, n0, p, w) awaiting sums

            def flush_one():
                e_t, n0, p, w = pending.pop(0)
                for j in range(p):
                    nc.tensor.matmul(
                        acc_ps[:, n0:N], i_sb[:], e_t[:, j, :],
                        start=False, stop=False, skip_group_check=True)
                if w > p:
                    nc.vector.tensor_reduce(
                        accn[:, n0:n0 + p], e_t[:, :, p:w],
                        mybir.AxisListType.X, A.add)

            for gi, (n0, p, w) in enumerate(groups):
                chunks = plans[gi]
                ps = psl1_pool.tile([B_LOCAL, p, w], F32)
                # seed l1 with SS[n] - SS[m]; selectors add 2*sum relu(d)
                nc.tensor.matmul(
                    ps[:], ir_sb[:],
                    ss_pos[:, n0:n0 + p].rearrange(
                        "b p -> b p ()").broadcast_to((B_LOCAL, p, w)),
                    start=True, stop=False)
                nc.tensor.matmul(
                    ps[:], ir_sb[:],
                    ss_neg[:, n0:N].rearrange(
                        "b w -> b () w").broadcast_to((B_LOCAL, p, w)),
                    start=False, stop=False)
                for c in range(N_CHUNKS):
                    eng = chunks[c]
                    r = r_pool.tile([128, p, w], BF16, tag="r")
                    for j in range(p):
                        n = n0 + j
                        if eng == "dve":
                            nc.vector.tensor_scalar(
                                r[:, j, :], mt_b[c][:, n0:N],
                                mt_f[c][:, n:n + 1], 0.0,
                                A.subtract, A.max)
                        elif eng == "act":
                            nc.scalar.activation(
                                r[:, j, :], mt_b[c][:, n0:N], Relu,
                                bias=mtn_f[c][:, n:n + 1], scale=1.0)
                        else:
                            bc = mt_b[c][:, n:n + 1].broadcast_to((128, w))
                            tmp = tmp_pool.tile([128, w], BF16, tag="tmp")
                            nc.gpsimd.tensor_tensor(
                                tmp[:], mt_b[c][:, n0:N], bc, A.subtract)
                            nc.gpsimd.tensor_scalar(
                                r[:, j, :], tmp[:], 0.0, None, A.max)
                    nc.tensor.matmul(
                        ps[:], w2_sb[:, B_LOCAL * c:B_LOCAL * (c + 1)],
                        r[:], start=False, stop=(c == N_CHUNKS - 1))
                e = e_pool.tile([B_LOCAL, p, w], BF16, tag="e")
                nc.scalar.activation(e[:], ps[:], Exp, bias=0.0, scale=-1.0)
                pending.append((e, n0, p, w))
                if len(pending) > LAG:
                    flush_one()
            while pending:
                flush_one()

            # ---- combine halves and store ----
            accf = e_pool.tile([B_LOCAL, N], F32, tag="accf")
            nc.vector.tensor_tensor(accf[:], accn[:], acc_ps[:], A.add)
            nc.sync.dma_start(out_d[:], accf[:])
            ap_.__exit__(None, None, None)
            lp.__exit__(None, None, None)

    _split_multi_waits(nc)
    return nc


def _selector(scale: float) -> np.ndarray:
    w = np.zeros((128, N_CHUNKS, B_LOCAL), dtype=np.float32)
    for c in range(N_CHUNKS):
        for p in range(128):
            w[p, c, (128 * c + p) // K] = scale
    return w.reshape(128, N_CHUNKS * B_LOCAL).astype(ml_dtypes.bfloat16)


def _in_maps(x: np.ndarray, T: np.ndarray) -> list:
    xt = np.ascontiguousarray(x.T)                       # (1024, 256)
    w = _selector(1.0)
    w2 = _selector(2.0)
    eye = np.eye(B_LOCAL, dtype=np.float32).astype(ml_dtypes.bfloat16)
    in_maps = []
    for c in range(N_CORES):
        tsh = np.ascontiguousarray(
            T[:, c * B_LOCAL:(c + 1) * B_LOCAL, :].reshape(IN_FEATURES, BK))
        in_maps.append({"xT": xt, "Tsh": tsh, "W": w, "W2": w2, "I32": eye,
                        "I32R": np.eye(B_LOCAL, dtype=np.float32)})
    return in_maps


def kernel(x: np.ndarray, T: np.ndarray) -> np.ndarray:
    global _COMPILED
    from concourse.bass_utils import run_bass_kernel_spmd

    x = np.ascontiguousarray(x, dtype=np.float32)
    T = np.ascontiguousarray(T, dtype=np.float32)

    if _COMPILED is None:
        _COMPILED = _build()
    nc = _COMPILED

    res = run_bass_kernel_spmd(nc, _in_maps(x, T), core_ids=list(range(N_CORES)))

    out = np.empty((N, IN_FEATURES + B_EXTRA), dtype=np.float32)
    out[:, :IN_FEATURES] = x
    for c in range(N_CORES):
        blk = res.results[c]["out"]                      # (32, 256) = (b, m)
        out[:, IN_FEATURES + c * B_LOCAL:IN_FEATURES + (c + 1) * B_LOCAL] = blk.T
    return out


# revision 16
# speedup vs baseline: 1.2293x; 1.0675x over previous
"""Trainium2 kernel for MinibatchDiscrimination.

reference:
    M = einsum('ni,ibk->nbk', x, T)            # (256, 256, 16)
    l1[n,m,b] = sum_k |M[n,b,k] - M[m,b,k]|
    out[m,b]  = sum_n exp(-l1[n,m,b]) - 1      # (256, 256)
    return concat([x, out], axis=1)            # (256, 1280)

Sharding: tensor-parallel over the B_extra=256 feature dim -> 32 features
per core, no collectives. Each core computes out[:, shard] as [32, 256]
(batch on partitions), host transposes and concatenates with x.

Per-core dataflow (block-triangle; E[n,m] symmetric):
  MT[(b,k), m] = M[m, b, k] via PE fp32r matmuls, (b,k) in 4 chunks of 128.
  Groups of p rows starting at n0 (p ~ 512/w, w = 256-n0) cover rows
  [n0, n0+p) x cols [n0, 256). Per group, one of two diff modes chosen by
  a static greedy balance over the engine cost model:
    abs mode: |M[:, m] - M[:, n]| per (row, chunk) -- ONE dual-op
        tensor_scalar (subtract, abs_max) at 4x on DVE, or an Abs
        activation (per-partition bias) on ACT; 4 selector matmuls (W)
        contract the (b,k) partitions into PSUM = exact l1.
    relu mode (GPSIMD): relu(d) via TT-subtract + TS-max (the only ALU
        ops walrus allows on Pool); PSUM pre-seeded with SS[n] - SS[m]
        (two fp32r matmuls with step-0 moving APs; SS = selector-contracted
        M, exact in f32) and selectors use W2 = 2W, since
        sum|d| = 2*sum relu(d) - SS[m] + SS[n].
  ACT: ONE exp per group over the whole [32, p, w] block (bias-free).
  PE: p identity passthrough-matmuls accumulate column sums into a
      PSUM-resident acc (memset to -1 cancels the diagonal's exp(0)=1
      against the final "-1").
  DVE: one tensor_reduce per group row-sums the beyond-the-pack columns
      into accn[:, n0:n0+p] (the mirrored cross-group triangle half).
  out_dev[b, m] = acc + accn, DMA'd [32, 256].

Each unordered pair lands exactly once: n<m via the colsum of n's group,
n>m cross-group via the rowsum of m's group (E symmetric), intra-pack
pairs both orders via the colsum of the shared group.
"""

import sys

sys.path.insert(0, "/opt/trn_rl_repo")

import os
import numpy as np
import ml_dtypes

LAG = int(os.environ.get("MBD_LAG", "5"))
R_BUFS = int(os.environ.get("MBD_R_BUFS", "36"))
E_BUFS = int(os.environ.get("MBD_E_BUFS", "9"))
PSL1_BUFS = int(os.environ.get("MBD_PSL1_BUFS", "7"))
PSMT_BUFS = int(os.environ.get("MBD_PSMT_BUFS", "2"))
# cost-model weights (ns) for the static assignment; tune empirically
C_DVE_I = float(os.environ.get("MBD_C_DVE_I", "85"))
C_DVE_W = float(os.environ.get("MBD_C_DVE_W", "0.54"))
C_ACT_I = float(os.environ.get("MBD_C_ACT_I", "225"))
C_ACT_W = float(os.environ.get("MBD_C_ACT_W", "0.833"))
C_POOL_I = float(os.environ.get("MBD_C_POOL_I", "700"))
C_POOL_W = float(os.environ.get("MBD_C_POOL_W", "1e6"))
DVE_FIXED = float(os.environ.get("MBD_DVE_FIXED", "4000"))
ACT_FIXED = float(os.environ.get("MBD_ACT_FIXED", "0"))
POOL_FIXED = float(os.environ.get("MBD_POOL_FIXED", "0"))
PE_FIXED = float(os.environ.get("MBD_PE_FIXED", "4500"))

N = 256
IN_FEATURES = 1024
B_EXTRA = 256
K = 16
N_CORES = 8
B_LOCAL = B_EXTRA // N_CORES          # 32 features per core
BK = B_LOCAL * K                      # 512 = (b_local, k) flattened
N_CHUNKS = BK // 128                  # 4 partition chunks of (b,k)
I_CHUNKS = IN_FEATURES // 128         # 8 contraction chunks

_COMPILED = None


def _apply_tile_drain_patch():
    """walrus in this container caps Drain (CTRL) instructions at one sem
    wait; Tile's end-of-kernel drain carries one wait per outstanding proc.
    Split the waits across a chain of drains."""
    from concourse import mybir, tile
    from concourse.vector_clock import ScopedClock

    def _drain_and_barrier(self, tick_clock, wait_clock):
        drain_inst = self.nc.sync.drain()
        wait_clock.add_sem_waits(
            drain_inst.ins, ScopedClock({None: tick_clock.global_clock})
        )
        si = drain_inst.ins.sync_info
        if si is not None and si.on_wait and len(si.on_wait) > 1:
            waits = list(si.on_wait)
            drain_inst.ins.sync_info = mybir.SyncInfo(
                on_wait=[waits[0]], on_update=list(si.on_update or [])
            )
            for w in waits[1:]:
                d = self.nc.sync.drain()
                d.ins.sync_info = mybir.SyncInfo(on_wait=[w], on_update=[])

        self.nc.all_engine_barrier()
        assert self.sems is not None
        popped = self.nc._tile_sem_poison_stack.pop()
        assert popped is self._sem_poison
        self.nc.clear_and_free_semaphores(list(self.sems.allocated().values()))
        self.nc.all_engine_barrier()

    tile.TileContext._drain_and_barrier = _drain_and_barrier


def _split_multi_waits(nc, max_waits=1):
    """This walrus build accepts at most one sync wait per instruction.
    Hoist extra waits onto NoOp instructions inserted just before the
    offending instruction in the same engine's stream."""
    from concourse import mybir

    cnt = 0
    for blk in nc.main_func.blocks:
        insts = blk.instructions
        if not any(
            inst.sync_info is not None
            and inst.sync_info.on_wait
            and len(inst.sync_info.on_wait) > max_waits
            for inst in insts
        ):
            continue
        new_list = []
        for inst in insts:
            si = inst.sync_info
            if si is not None and si.on_wait and len(si.on_wait) > max_waits:
                waits = list(si.on_wait)
                for w in waits[:-max_waits]:
                    nop = mybir.InstNoOp(name=f"wsplit-{cnt}", ins=[], outs=[])
                    cnt += 1
                    nop.engine = inst.engine
                    nop.sync_info = mybir.SyncInfo(on_wait=[w], on_update=[])
                    new_list.append(nop)
                inst.sync_info = mybir.SyncInfo(
                    on_wait=waits[-max_waits:],
                    on_update=list(si.on_update or []),
                )
            new_list.append(inst)
        insts[:] = new_list
    return cnt


def _groups():
    gs = []
    n0 = 0
    while n0 < N:
        w = N - n0
        p = 2 * (256 // w)
        p = min(p, w)
        gs.append((n0, p, w))
        n0 += p
    return gs


def _assign_diffs(groups):
    """Static greedy balance (ns, empirically calibrated against the
    simulator): per-(group, chunk) diff engine in {dve, act}, plus a
    per-group column-sum engine in {pe, pool}. PE matmuls are costed at
    mid p-state (0.87 ns/row)."""
    pe_mm = 0.87
    loads = {
        "dve": DVE_FIXED + sum(
            (58 + p * (w - p)) * 1.0417 for (_, p, w) in groups if w > p),
        "act": ACT_FIXED + sum(
            210.0 + 0.833 * p * w for (_, p, w) in groups),
        "pool": POOL_FIXED,
        "pe": PE_FIXED + sum(6 * p * w * pe_mm for (_, p, w) in groups),
    }
    cost = {
        "dve": lambda p, w: p * (C_DVE_I + C_DVE_W * w),
        "act": lambda p, w: p * (C_ACT_I + C_ACT_W * w),
    }
    plans = []
    for (n0, p, w) in groups:
        chunks = []
        for c in range(N_CHUNKS):
            best = min(cost, key=lambda e: loads[e] + cost[e](p, w))
            loads[best] += cost[best](p, w)
            chunks.append(best)
        c_pe = p * w * pe_mm
        c_pool = p * (2.1 * w + 60.0)
        if loads["pe"] + c_pe <= loads["pool"] + c_pool:
            loads["pe"] += c_pe
            cs = "pe"
        else:
            loads["pool"] += c_pool
            cs = "pool"
        plans.append((chunks, cs))
    return plans, loads


def _build():
    from concourse import bass, mybir, tile

    _apply_tile_drain_patch()
    A = mybir.AluOpType
    F32 = mybir.dt.float32
    F32R = mybir.dt.float32r
    BF16 = mybir.dt.bfloat16

    nc = bass.Bass()
    xt_d = nc.declare_dram_parameter("xT", [IN_FEATURES, N], F32R, isOutput=False)
    t_d = nc.declare_dram_parameter("Tsh", [IN_FEATURES, BK], F32R, isOutput=False)
    w_d = nc.declare_dram_parameter("W", [128, N_CHUNKS * B_LOCAL], BF16,
                                    isOutput=False)
    w2_d = nc.declare_dram_parameter("W2", [128, N_CHUNKS * B_LOCAL], BF16,
                                     isOutput=False)
    i_d = nc.declare_dram_parameter("I32", [B_LOCAL, B_LOCAL], BF16,
                                    isOutput=False)
    ir_d = nc.declare_dram_parameter("I32R", [B_LOCAL, B_LOCAL], F32R,
                                     isOutput=False)
    out_d = nc.declare_dram_parameter("out", [B_LOCAL, N], F32, isOutput=True)

    groups = _groups()
    plans, loads = _assign_diffs(groups)

    Exp = mybir.ActivationFunctionType.Exp
    Relu = mybir.ActivationFunctionType.Relu

    with tile.TileContext(nc) as tc:
        with (
            tc.tile_pool(name="const", bufs=1) as const_pool,
            tc.tile_pool(name="mt", bufs=1) as mt_pool,
            tc.tile_pool(name="r", bufs=R_BUFS) as r_pool,
            tc.tile_pool(name="tmp", bufs=4) as tmp_pool,
            tc.tile_pool(name="e", bufs=E_BUFS) as e_pool,
        ):
            # ---- load inputs (split so MT matmuls can overlap the DMA) ----
            xt = const_pool.tile([128, I_CHUNKS, N], F32R, tag="xt")
            for ic in range(I_CHUNKS):
                nc.sync.dma_start(
                    xt[:, ic, :], xt_d[128 * ic:128 * (ic + 1), :])
            tsh = const_pool.tile([128, I_CHUNKS, BK], F32R, tag="tsh")
            for ic in range(I_CHUNKS):
                nc.sync.dma_start(
                    tsh[:, ic, :], t_d[128 * ic:128 * (ic + 1), :])
            w_sb = const_pool.tile([128, N_CHUNKS * B_LOCAL], BF16, tag="w")
            nc.sync.dma_start(w_sb[:], w_d[:])
            w2_sb = const_pool.tile([128, N_CHUNKS * B_LOCAL], BF16, tag="w2")
            nc.sync.dma_start(w2_sb[:], w2_d[:])
            i_sb = const_pool.tile([B_LOCAL, B_LOCAL], BF16, tag="i32")
            nc.sync.dma_start(i_sb[:], i_d[:])
            ir_sb = const_pool.tile([B_LOCAL, B_LOCAL], F32R, tag="i32r")
            nc.sync.dma_start(ir_sb[:], ir_d[:])

            mt_psum = tc.tile_pool(name="psmt", bufs=PSMT_BUFS, space="PSUM")
            psmt_pool = mt_psum.__enter__()
            ss_psum = tc.tile_pool(name="psss", bufs=1, space="PSUM")
            psss_pool = ss_psum.__enter__()
            # ---- MT[(b,k), m] per chunk: bf16 stream + f32 scalar copies ----
            # mt_f = f32(upcast(bf16(M))) so the diff diagonal is exactly 0.
            mt_b, mt_f, mtn_f = [], [], []
            for c in range(N_CHUNKS):
                ps = psmt_pool.tile([128, N], F32)
                for ic in range(I_CHUNKS):
                    nc.tensor.matmul(
                        ps[:],
                        tsh[:, ic, 128 * c:128 * (c + 1)],
                        xt[:, ic, :],
                        start=(ic == 0),
                        stop=(ic == I_CHUNKS - 1),
                    )
                mb = mt_pool.tile([128, N], BF16, tag=f"mtb{c}")
                nc.vector.tensor_copy(mb[:], ps[:])
                mf = mt_pool.tile([128, N], F32, tag=f"mtf{c}")
                nc.vector.tensor_copy(mf[:], mb[:])
                nf = mt_pool.tile([128, N], F32, tag=f"mtnf{c}")
                nc.vector.tensor_scalar(nf[:], mb[:], -1.0, None, A.mult)
                mt_b.append(mb)
                mt_f.append(mf)
                mtn_f.append(nf)

            # ---- SS[b, m] = sum_k bf16(M)[m, b, k] for the l1 seeds ----
            if True:
                ss_ps = psss_pool.tile([B_LOCAL, N], F32, tag="ssps")
                for c in range(N_CHUNKS):
                    nc.tensor.matmul(
                        ss_ps[:], w_sb[:, B_LOCAL * c:B_LOCAL * (c + 1)],
                        mt_b[c][:], start=(c == 0), stop=(c == N_CHUNKS - 1))
                ss_pos = mt_pool.tile([B_LOCAL, N], F32R, tag="sspos")
                nc.vector.tensor_copy(ss_pos[:], ss_ps[:])
                ss_neg = mt_pool.tile([B_LOCAL, N], F32R, tag="ssneg")
                nc.vector.tensor_scalar(
                    ss_neg[:], ss_ps[:], -1.0, None, A.mult)

            ss_psum.__exit__(None, None, None)
            mt_psum.__exit__(None, None, None)
            any_pe_cs = any(cs == "pe" for _, cs in plans)
            lp = tc.tile_pool(name="psl1", bufs=PSL1_BUFS, space="PSUM")
            psl1_pool = lp.__enter__()
            ap_ = tc.tile_pool(name="psacc", bufs=1, space="PSUM") if any_pe_cs else None
            psacc_pool = ap_.__enter__() if any_pe_cs else None
            # ---- accumulators ----
            accn = e_pool.tile([B_LOCAL, N], F32, tag="accn")
            nc.gpsimd.memset(accn[:], 0.0)
            acc_sb = e_pool.tile([B_LOCAL, N], F32, tag="accsb")
            nc.gpsimd.memset(acc_sb[:], -1.0)   # cancels the diagonal exp(0)=1
            if any_pe_cs:
                acc_ps = psacc_pool.tile([B_LOCAL, N], F32)
                nc.vector.memset(acc_ps[:], 0.0)

            # ---- main loop ----
            pending = []                        # (e, n0, p, w) awaiting sums

            def flush_one():
                e_t, n0, p, w, cs = pending.pop(0)
                for j in range(p):
                    if cs == "pe":
                        nc.tensor.matmul(
                            acc_ps[:, n0:N], i_sb[:], e_t[:, j, :],
                            start=False, stop=False, skip_group_check=True)
                    else:
                        nc.gpsimd.tensor_tensor(
                            acc_sb[:, n0:N], acc_sb[:, n0:N], e_t[:, j, :],
                            A.add)
                if w > p:
                    nc.vector.tensor_reduce(
                        accn[:, n0:n0 + p], e_t[:, :, p:w],
                        mybir.AxisListType.X, A.add)

            for gi, (n0, p, w) in enumerate(groups):
                chunks, _cs = plans[gi]
                ps = psl1_pool.tile([B_LOCAL, p, w], F32)
                # seed l1 with SS[n] - SS[m]; selectors add 2*sum relu(d)
                nc.tensor.matmul(
                    ps[:], ir_sb[:],
                    ss_pos[:, n0:n0 + p].rearrange(
                        "b p -> b p ()").broadcast_to((B_LOCAL, p, w)),
                    start=True, stop=False)
                nc.tensor.matmul(
                    ps[:], ir_sb[:],
                    ss_neg[:, n0:N].rearrange(
                        "b w -> b () w").broadcast_to((B_LOCAL, p, w)),
                    start=False, stop=False)
                for c in range(N_CHUNKS):
                    eng = chunks[c]
                    r = r_pool.tile([128, p, w], BF16, tag="r")
                    for j in range(p):
                        n = n0 + j
                        if eng == "dve":
                            nc.vector.tensor_scalar(
                                r[:, j, :], mt_b[c][:, n0:N],
                                mt_f[c][:, n:n + 1], 0.0,
                                A.subtract, A.max)
                        elif eng == "act":
                            nc.scalar.activation(
                                r[:, j, :], mt_b[c][:, n0:N], Relu,
                                bias=mtn_f[c][:, n:n + 1], scale=1.0)
                        else:
                            bc = mt_b[c][:, n:n + 1].broadcast_to((128, w))
                            tmp = tmp_pool.tile([128, w], BF16, tag="tmp")
                            nc.gpsimd.tensor_tensor(
                                tmp[:], mt_b[c][:, n0:N], bc, A.subtract)
                            nc.gpsimd.tensor_scalar(
                                r[:, j, :], tmp[:], 0.0, None, A.max)
                    nc.tensor.matmul(
                        ps[:], w2_sb[:, B_LOCAL * c:B_LOCAL * (c + 1)],
                        r[:], start=False, stop=(c == N_CHUNKS - 1))
                e = e_pool.tile([B_LOCAL, p, w], BF16, tag="e")
                nc.scalar.activation(e[:], ps[:], Exp, bias=0.0, scale=-1.0)
                pending.append((e, n0, p, w, plans[gi][1]))
                if len(pending) > LAG:
                    flush_one()
            while pending:
                flush_one()

            # ---- combine halves and store ----
            accf = e_pool.tile([B_LOCAL, N], F32, tag="accf")
            nc.vector.tensor_tensor(accf[:], accn[:], acc_sb[:], A.add)
            if any_pe_cs:
                nc.vector.tensor_tensor(accf[:], accf[:], acc_ps[:], A.add)
            nc.sync.dma_start(out_d[:], accf[:])
            if ap_ is not None:
                ap_.__exit__(None, None, None)
            lp.__exit__(None, None, None)

    _split_multi_waits(nc)
    return nc


def _selector(scale: float) -> np.ndarray:
    w = np.zeros((128, N_CHUNKS, B_LOCAL), dtype=np.float32)
    for c in range(N_CHUNKS):
        for p in range(128):
            w[p, c, (128 * c + p) // K] = scale
    return w.reshape(128, N_CHUNKS * B_LOCAL).astype(ml_dtypes.bfloat16)


def _in_maps(x: np.ndarray, T: np.ndarray) -> list:
    xt = np.ascontiguousarray(x.T)                       # (1024, 256)
    w = _selector(1.0)
    w2 = _selector(2.0)
    eye = np.eye(B_LOCAL, dtype=np.float32).astype(ml_dtypes.bfloat16)
    in_maps = []
    for c in range(N_CORES):
        tsh = np.ascontiguousarray(
            T[:, c * B_LOCAL:(c + 1) * B_LOCAL, :].reshape(IN_FEATURES, BK))
        in_maps.append({"xT": xt, "Tsh": tsh, "W": w, "W2": w2, "I32": eye,
                        "I32R": np.eye(B_LOCAL, dtype=np.float32)})
    return in_maps


def kernel(x: np.ndarray, T: np.ndarray) -> np.ndarray:
    global _COMPILED
    from concourse.bass_utils import run_bass_kernel_spmd

    x = np.ascontiguousarray(x, dtype=np.float32)
    T = np.ascontiguousarray(T, dtype=np.float32)

    if _COMPILED is None:
        _COMPILED = _build()
    nc = _COMPILED

    res = run_bass_kernel_spmd(nc, _in_maps(x, T), core_ids=list(range(N_CORES)))

    out = np.empty((N, IN_FEATURES + B_EXTRA), dtype=np.float32)
    out[:, :IN_FEATURES] = x
    for c in range(N_CORES):
        blk = res.results[c]["out"]                      # (32, 256) = (b, m)
        out[:, IN_FEATURES + c * B_LOCAL:IN_FEATURES + (c + 1) * B_LOCAL] = blk.T
    return out


# revision 21
# speedup vs baseline: 1.2309x; 1.0013x over previous
"""Trainium2 kernel for MinibatchDiscrimination.

reference:
    M = einsum('ni,ibk->nbk', x, T)            # (256, 256, 16)
    l1[n,m,b] = sum_k |M[n,b,k] - M[m,b,k]|
    out[m,b]  = sum_n exp(-l1[n,m,b]) - 1      # (256, 256)
    return concat([x, out], axis=1)            # (256, 1280)

Sharding: tensor-parallel over the B_extra=256 feature dim -> 32 features
per core, no collectives. Each core computes out[:, shard] as [32, 256]
(batch on partitions), host transposes and concatenates with x.

Per-core dataflow (block-triangle; E[n,m] symmetric):
  MT[(b,k), m] = M[m, b, k] via PE fp32r matmuls, (b,k) in 4 chunks of 128.
  Groups of p rows starting at n0 (p ~ 512/w, w = 256-n0) cover rows
  [n0, n0+p) x cols [n0, 256). Per group, one of two diff modes chosen by
  a static greedy balance over the engine cost model:
    abs mode: |M[:, m] - M[:, n]| per (row, chunk) -- ONE dual-op
        tensor_scalar (subtract, abs_max) at 4x on DVE, or an Abs
        activation (per-partition bias) on ACT; 4 selector matmuls (W)
        contract the (b,k) partitions into PSUM = exact l1.
    relu mode (GPSIMD): relu(d) via TT-subtract + TS-max (the only ALU
        ops walrus allows on Pool); PSUM pre-seeded with SS[n] - SS[m]
        (two fp32r matmuls with step-0 moving APs; SS = selector-contracted
        M, exact in f32) and selectors use W2 = 2W, since
        sum|d| = 2*sum relu(d) - SS[m] + SS[n].
  ACT: ONE exp per group over the whole [32, p, w] block (bias-free).
  PE: p identity passthrough-matmuls accumulate column sums into a
      PSUM-resident acc (memset to -1 cancels the diagonal's exp(0)=1
      against the final "-1").
  DVE: one tensor_reduce per group row-sums the beyond-the-pack columns
      into accn[:, n0:n0+p] (the mirrored cross-group triangle half).
  out_dev[b, m] = acc + accn, DMA'd [32, 256].

Each unordered pair lands exactly once: n<m via the colsum of n's group,
n>m cross-group via the rowsum of m's group (E symmetric), intra-pack
pairs both orders via the colsum of the shared group.
"""

import sys

sys.path.insert(0, "/opt/trn_rl_repo")

import os
import numpy as np
import ml_dtypes

LAG = int(os.environ.get("MBD_LAG", "5"))
R_BUFS = int(os.environ.get("MBD_R_BUFS", "36"))
E_BUFS = int(os.environ.get("MBD_E_BUFS", "9"))
PSL1_BUFS = int(os.environ.get("MBD_PSL1_BUFS", "7"))
PSMT_BUFS = int(os.environ.get("MBD_PSMT_BUFS", "2"))
# cost-model weights (ns) for the static assignment; tune empirically
C_DVE_I = float(os.environ.get("MBD_C_DVE_I", "85"))
C_DVE_W = float(os.environ.get("MBD_C_DVE_W", "0.54"))
C_ACT_I = float(os.environ.get("MBD_C_ACT_I", "225"))
C_ACT_W = float(os.environ.get("MBD_C_ACT_W", "0.833"))
C_POOL_I = float(os.environ.get("MBD_C_POOL_I", "700"))
C_POOL_W = float(os.environ.get("MBD_C_POOL_W", "1e6"))
DVE_FIXED = float(os.environ.get("MBD_DVE_FIXED", "4000"))
ACT_FIXED = float(os.environ.get("MBD_ACT_FIXED", "0"))
POOL_FIXED = float(os.environ.get("MBD_POOL_FIXED", "0"))
PE_FIXED = float(os.environ.get("MBD_PE_FIXED", "4500"))

N = 256
IN_FEATURES = 1024
B_EXTRA = 256
K = 16
N_CORES = 8
B_LOCAL = B_EXTRA // N_CORES          # 32 features per core
BK = B_LOCAL * K                      # 512 = (b_local, k) flattened
N_CHUNKS = BK // 128                  # 4 partition chunks of (b,k)
I_CHUNKS = IN_FEATURES // 128         # 8 contraction chunks

_COMPILED = None


def _apply_tile_drain_patch():
    """walrus in this container caps Drain (CTRL) instructions at one sem
    wait; Tile's end-of-kernel drain carries one wait per outstanding proc.
    Split the waits across a chain of drains."""
    from concourse import mybir, tile
    from concourse.vector_clock import ScopedClock

    def _drain_and_barrier(self, tick_clock, wait_clock):
        drain_inst = self.nc.sync.drain()
        wait_clock.add_sem_waits(
            drain_inst.ins, ScopedClock({None: tick_clock.global_clock})
        )
        si = drain_inst.ins.sync_info
        if si is not None and si.on_wait and len(si.on_wait) > 1:
            waits = list(si.on_wait)
            drain_inst.ins.sync_info = mybir.SyncInfo(
                on_wait=[waits[0]], on_update=list(si.on_update or [])
            )
            for w in waits[1:]:
                d = self.nc.sync.drain()
                d.ins.sync_info = mybir.SyncInfo(on_wait=[w], on_update=[])

        self.nc.all_engine_barrier()
        assert self.sems is not None
        popped = self.nc._tile_sem_poison_stack.pop()
        assert popped is self._sem_poison
        self.nc.clear_and_free_semaphores(list(self.sems.allocated().values()))
        self.nc.all_engine_barrier()

    tile.TileContext._drain_and_barrier = _drain_and_barrier


def _split_multi_waits(nc, max_waits=1):
    """This walrus build accepts at most one sync wait per instruction.
    Hoist extra waits onto NoOp instructions inserted just before the
    offending instruction in the same engine's stream."""
    from concourse import mybir

    cnt = 0
    for blk in nc.main_func.blocks:
        insts = blk.instructions
        if not any(
            inst.sync_info is not None
            and inst.sync_info.on_wait
            and len(inst.sync_info.on_wait) > max_waits
            for inst in insts
        ):
            continue
        new_list = []
        for inst in insts:
            si = inst.sync_info
            if si is not None and si.on_wait and len(si.on_wait) > max_waits:
                waits = list(si.on_wait)
                for w in waits[:-max_waits]:
                    nop = mybir.InstNoOp(name=f"wsplit-{cnt}", ins=[], outs=[])
                    cnt += 1
                    nop.engine = inst.engine
                    nop.sync_info = mybir.SyncInfo(on_wait=[w], on_update=[])
                    new_list.append(nop)
                inst.sync_info = mybir.SyncInfo(
                    on_wait=waits[-max_waits:],
                    on_update=list(si.on_update or []),
                )
            new_list.append(inst)
        insts[:] = new_list
    return cnt


def _groups():
    gs = []
    n0 = 0
    while n0 < N:
        w = N - n0
        p = max(2, 2 * (256 // w))
        p = min(p, w)
        gs.append((n0, p, w))
        n0 += p
    return gs


def _assign_diffs(groups):
    """Static greedy balance (ns, empirically calibrated against the
    simulator). Per group: diff engine per chunk in {dve, act}; column-sum
    engine in {pe, pool}; and for p==2 groups an exp mode in {group,
    perrow} -- perrow replaces the +SS[n] seed matmul and the DVE row-sum
    reduce with two bias'd accum_out exps on ACT."""
    pe_mm = 0.87
    loads = {"dve": DVE_FIXED, "act": ACT_FIXED, "pool": POOL_FIXED,
             "pe": PE_FIXED + sum(6 * p * w * pe_mm for (_, p, w) in groups)}
    cost = {
        "dve": lambda p, w: p * (C_DVE_I + C_DVE_W * w),
        "act": lambda p, w: p * (C_ACT_I + C_ACT_W * w),
    }
    plans = []
    for (n0, p, w) in groups:
        chunks = []
        for c in range(N_CHUNKS):
            best = min(cost, key=lambda e: loads[e] + cost[e](p, w))
            loads[best] += cost[best](p, w)
            chunks.append(best)
        # column-sum engine
        c_pe = p * w * pe_mm
        c_pool = p * (60.0 + 3.0 * w)
        if loads["pe"] + c_pe <= loads["pool"] + c_pool:
            loads["pe"] += c_pe
            cs = "pe"
        else:
            loads["pool"] += c_pool
            cs = "pool"
        # exp mode
        act_group = 210.0 + 0.833 * p * w
        dve_rsum = (58 + p * (w - p)) * 1.0417 if w > p else 0.0
        if p == 2:
            act_perrow = 2 * (397.0 + 0.833 * (w - 2)) + 2 * 212.0
            mk_g = max(loads["act"] + act_group, loads["dve"] + dve_rsum,
                       loads["pe"])
            mk_p = max(loads["act"] + act_perrow, loads["dve"],
                       loads["pe"] - p * w * pe_mm)
            if mk_p < mk_g:
                loads["act"] += act_perrow
                loads["pe"] -= p * w * pe_mm
                plans.append((chunks, cs, "perrow"))
                continue
        loads["act"] += act_group
        loads["dve"] += dve_rsum
        plans.append((chunks, cs, "group"))
    return plans, loads


def _build():
    from concourse import bass, mybir, tile

    _apply_tile_drain_patch()
    A = mybir.AluOpType
    F32 = mybir.dt.float32
    F32R = mybir.dt.float32r
    BF16 = mybir.dt.bfloat16

    nc = bass.Bass()
    xt_d = nc.declare_dram_parameter("xT", [IN_FEATURES, N], F32R, isOutput=False)
    t_d = nc.declare_dram_parameter("Tsh", [IN_FEATURES, BK], F32R, isOutput=False)
    w_d = nc.declare_dram_parameter("W", [128, N_CHUNKS * B_LOCAL], BF16,
                                    isOutput=False)
    w2_d = nc.declare_dram_parameter("W2", [128, N_CHUNKS * B_LOCAL], BF16,
                                     isOutput=False)
    i_d = nc.declare_dram_parameter("I32", [B_LOCAL, B_LOCAL], BF16,
                                    isOutput=False)
    ir_d = nc.declare_dram_parameter("I32R", [B_LOCAL, B_LOCAL], F32R,
                                     isOutput=False)
    out_d = nc.declare_dram_parameter("out", [B_LOCAL, N], F32, isOutput=True)

    groups = _groups()
    plans, loads = _assign_diffs(groups)

    Exp = mybir.ActivationFunctionType.Exp
    Relu = mybir.ActivationFunctionType.Relu

    with tile.TileContext(nc) as tc:
        with (
            tc.tile_pool(name="const", bufs=1) as const_pool,
            tc.tile_pool(name="mt", bufs=1) as mt_pool,
            tc.tile_pool(name="r", bufs=R_BUFS) as r_pool,
            tc.tile_pool(name="tmp", bufs=4) as tmp_pool,
            tc.tile_pool(name="e", bufs=E_BUFS) as e_pool,
        ):
            # ---- load inputs (split so MT matmuls can overlap the DMA) ----
            xt = const_pool.tile([128, I_CHUNKS, N], F32R, tag="xt")
            for ic in range(I_CHUNKS):
                nc.sync.dma_start(
                    xt[:, ic, :], xt_d[128 * ic:128 * (ic + 1), :])
            tsh = const_pool.tile([128, I_CHUNKS, BK], F32R, tag="tsh")
            for ic in range(I_CHUNKS):
                nc.sync.dma_start(
                    tsh[:, ic, :], t_d[128 * ic:128 * (ic + 1), :])
            w_sb = const_pool.tile([128, N_CHUNKS * B_LOCAL], BF16, tag="w")
            nc.sync.dma_start(w_sb[:], w_d[:])
            w2_sb = const_pool.tile([128, N_CHUNKS * B_LOCAL], BF16, tag="w2")
            nc.sync.dma_start(w2_sb[:], w2_d[:])
            i_sb = const_pool.tile([B_LOCAL, B_LOCAL], BF16, tag="i32")
            nc.sync.dma_start(i_sb[:], i_d[:])
            ir_sb = const_pool.tile([B_LOCAL, B_LOCAL], F32R, tag="i32r")
            nc.sync.dma_start(ir_sb[:], ir_d[:])

            mt_psum = tc.tile_pool(name="psmt", bufs=PSMT_BUFS, space="PSUM")
            psmt_pool = mt_psum.__enter__()
            ss_psum = tc.tile_pool(name="psss", bufs=1, space="PSUM")
            psss_pool = ss_psum.__enter__()
            # ---- MT[(b,k), m] per chunk: bf16 stream + f32 scalar copies ----
            # mt_f = f32(upcast(bf16(M))) so the diff diagonal is exactly 0.
            mt_b, mt_f, mtn_f = [], [], []
            for c in range(N_CHUNKS):
                ps = psmt_pool.tile([128, N], F32)
                for ic in range(I_CHUNKS):
                    nc.tensor.matmul(
                        ps[:],
                        tsh[:, ic, 128 * c:128 * (c + 1)],
                        xt[:, ic, :],
                        start=(ic == 0),
                        stop=(ic == I_CHUNKS - 1),
                    )
                mb = mt_pool.tile([128, N], BF16, tag=f"mtb{c}")
                nc.vector.tensor_copy(mb[:], ps[:])
                mf = mt_pool.tile([128, N], F32, tag=f"mtf{c}")
                nc.vector.tensor_copy(mf[:], mb[:])
                nf = mt_pool.tile([128, N], F32, tag=f"mtnf{c}")
                nc.vector.tensor_scalar(nf[:], mb[:], -1.0, None, A.mult)
                mt_b.append(mb)
                mt_f.append(mf)
                mtn_f.append(nf)

            # ---- SS[b, m] = sum_k bf16(M)[m, b, k] for the l1 seeds ----
            if True:
                ss_ps = psss_pool.tile([B_LOCAL, N], F32, tag="ssps")
                for c in range(N_CHUNKS):
                    nc.tensor.matmul(
                        ss_ps[:], w_sb[:, B_LOCAL * c:B_LOCAL * (c + 1)],
                        mt_b[c][:], start=(c == 0), stop=(c == N_CHUNKS - 1))
                ss_pos = mt_pool.tile([B_LOCAL, N], F32R, tag="sspos")
                nc.vector.tensor_copy(ss_pos[:], ss_ps[:])
                ss_neg = mt_pool.tile([B_LOCAL, N], F32R, tag="ssneg")
                nc.vector.tensor_scalar(
                    ss_neg[:], ss_ps[:], -1.0, None, A.mult)
                ssneg_f = mt_pool.tile([B_LOCAL, N], F32, tag="ssnegf")
                nc.vector.tensor_scalar(
                    ssneg_f[:], ss_ps[:], -1.0, None, A.mult)

            ss_psum.__exit__(None, None, None)
            mt_psum.__exit__(None, None, None)
            any_pe_cs = any(cs == "pe" for _, cs, _ in plans)
            lp = tc.tile_pool(name="psl1", bufs=PSL1_BUFS, space="PSUM")
            psl1_pool = lp.__enter__()
            ap_ = tc.tile_pool(name="psacc", bufs=1, space="PSUM") if any_pe_cs else None
            psacc_pool = ap_.__enter__() if any_pe_cs else None
            # ---- accumulators ----
            accn = e_pool.tile([B_LOCAL, N], F32, tag="accn")
            nc.gpsimd.memset(accn[:], 0.0)
            acc_sb = e_pool.tile([B_LOCAL, N], F32, tag="accsb")
            nc.gpsimd.memset(acc_sb[:], -1.0)   # cancels the diagonal exp(0)=1
            if any_pe_cs:
                acc_ps = psacc_pool.tile([B_LOCAL, N], F32)
                nc.vector.memset(acc_ps[:], 0.0)

            # ---- main loop ----
            pending = []                        # (e, n0, p, w) awaiting sums

            def flush_one():
                e_t, n0, p, w, cs, emode = pending.pop(0)
                for j in range(p):
                    if cs == "pe":
                        nc.tensor.matmul(
                            acc_ps[:, n0:N], i_sb[:], e_t[:, j, :],
                            start=False, stop=False, skip_group_check=True)
                    else:
                        nc.gpsimd.tensor_tensor(
                            acc_sb[:, n0:N], acc_sb[:, n0:N], e_t[:, j, :],
                            A.add)
                if emode == "group" and w > p:
                    nc.vector.tensor_reduce(
                        accn[:, n0:n0 + p], e_t[:, :, p:w],
                        mybir.AxisListType.X, A.add)

            for gi, (n0, p, w) in enumerate(groups):
                chunks, _cs, emode = plans[gi]
                ps = psl1_pool.tile([B_LOCAL, p, w], F32)
                # seed l1 with SS[n] - SS[m]; selectors add 2*sum relu(d).
                # perrow groups fold the +SS[n] term into the exp bias.
                if emode == "group":
                    nc.tensor.matmul(
                        ps[:], ir_sb[:],
                        ss_pos[:, n0:n0 + p].rearrange(
                            "b p -> b p ()").broadcast_to((B_LOCAL, p, w)),
                        start=True, stop=False)
                nc.tensor.matmul(
                    ps[:], ir_sb[:],
                    ss_neg[:, n0:N].rearrange(
                        "b w -> b () w").broadcast_to((B_LOCAL, p, w)),
                    start=(emode != "group"), stop=False)
                for c in range(N_CHUNKS):
                    eng = chunks[c]
                    r = r_pool.tile([128, p, w], BF16, tag="r")
                    for j in range(p):
                        n = n0 + j
                        if eng == "dve":
                            nc.vector.tensor_scalar(
                                r[:, j, :], mt_b[c][:, n0:N],
                                mt_f[c][:, n:n + 1], 0.0,
                                A.subtract, A.max)
                        elif eng == "act":
                            nc.scalar.activation(
                                r[:, j, :], mt_b[c][:, n0:N], Relu,
                                bias=mtn_f[c][:, n:n + 1], scale=1.0)
                        else:
                            bc = mt_b[c][:, n:n + 1].broadcast_to((128, w))
                            tmp = tmp_pool.tile([128, w], BF16, tag="tmp")
                            nc.gpsimd.tensor_tensor(
                                tmp[:], mt_b[c][:, n0:N], bc, A.subtract)
                            nc.gpsimd.tensor_scalar(
                                r[:, j, :], tmp[:], 0.0, None, A.max)
                    nc.tensor.matmul(
                        ps[:], w2_sb[:, B_LOCAL * c:B_LOCAL * (c + 1)],
                        r[:], start=False, stop=(c == N_CHUNKS - 1))
                e = e_pool.tile([B_LOCAL, p, w], BF16, tag="e")
                if emode == "group":
                    nc.scalar.activation(
                        e[:], ps[:], Exp, bias=0.0, scale=-1.0)
                else:
                    # per-row exps: bias carries -SS[n]; the beyond-pack
                    # slice's accum_out emits the row sum (mirror half)
                    for j in range(p):
                        n = n0 + j
                        nc.scalar.activation(
                            e[:, j, 0:p], ps[:, j, 0:p], Exp,
                            bias=ssneg_f[:, n:n + 1], scale=-1.0)
                        nc.scalar.activation(
                            e[:, j, p:w], ps[:, j, p:w], Exp,
                            bias=ssneg_f[:, n:n + 1], scale=-1.0,
                            accum_out=accn[:, n:n + 1])
                pending.append((e, n0, p, w, plans[gi][1], emode))
                if len(pending) > LAG:
                    flush_one()
            while pending:
                flush_one()

            # ---- combine halves and store ----
            accf = e_pool.tile([B_LOCAL, N], F32, tag="accf")
            nc.vector.tensor_tensor(accf[:], accn[:], acc_sb[:], A.add)
            if any_pe_cs:
                nc.vector.tensor_tensor(accf[:], accf[:], acc_ps[:], A.add)
            nc.sync.dma_start(out_d[:], accf[:])
            if ap_ is not None:
                ap_.__exit__(None, None, None)
            lp.__exit__(None, None, None)

    _split_multi_waits(nc)
    return nc


def _selector(scale: float) -> np.ndarray:
    w = np.zeros((128, N_CHUNKS, B_LOCAL), dtype=np.float32)
    for c in range(N_CHUNKS):
        for p in range(128):
            w[p, c, (128 * c + p) // K] = scale
    return w.reshape(128, N_CHUNKS * B_LOCAL).astype(ml_dtypes.bfloat16)


def _in_maps(x: np.ndarray, T: np.ndarray) -> list:
    xt = np.ascontiguousarray(x.T)                       # (1024, 256)
    w = _selector(1.0)
    w2 = _selector(2.0)
    eye = np.eye(B_LOCAL, dtype=np.float32).astype(ml_dtypes.bfloat16)
    in_maps = []
    for c in range(N_CORES):
        tsh = np.ascontiguousarray(
            T[:, c * B_LOCAL:(c + 1) * B_LOCAL, :].reshape(IN_FEATURES, BK))
        in_maps.append({"xT": xt, "Tsh": tsh, "W": w, "W2": w2, "I32": eye,
                        "I32R": np.eye(B_LOCAL, dtype=np.float32)})
    return in_maps


def kernel(x: np.ndarray, T: np.ndarray) -> np.ndarray:
    global _COMPILED
    from concourse.bass_utils import run_bass_kernel_spmd

    x = np.ascontiguousarray(x, dtype=np.float32)
    T = np.ascontiguousarray(T, dtype=np.float32)

    if _COMPILED is None:
        _COMPILED = _build()
    nc = _COMPILED

    res = run_bass_kernel_spmd(nc, _in_maps(x, T), core_ids=list(range(N_CORES)))

    out = np.empty((N, IN_FEATURES + B_EXTRA), dtype=np.float32)
    out[:, :IN_FEATURES] = x
    for c in range(N_CORES):
        blk = res.results[c]["out"]                      # (32, 256) = (b, m)
        out[:, IN_FEATURES + c * B_LOCAL:IN_FEATURES + (c + 1) * B_LOCAL] = blk.T
    return out
